# revision 3
# baseline (speedup 1.0000x reference)
"""Trainium2 Bass kernel for the attention-pooling layer (fp16 pipeline),
data-parallel over batch with replicated fc_w, cross-rep software
pipelined.

Computation (per sample b):
    q = input2 @ fc_w.T + fc_b                      # [B, C1]
    scores[b, p] = <input1[b, :, p], q[b]>          # [B, HW]
    attn = softmax(scores, axis=1)
    out[b, c] = sum_p input1[b, c, p] * attn[b, p]  # [B, C1]

Sharding: x is data-parallel over batch (8 samples/core), fc_w
replicated -- measured A/B: an AllToAll/ReduceScatter q-exchange with
sharded fc_w costs ~12us/~90us of hard-serialized collective time per
rep on this stack, far more than the 3.7MB of HBM it saves (the kernel
is compute-bound at ~63us/rep, DMA is 47.9us).  q for the core's own 8
samples is computed DIRECTLY TRANSPOSED: per c1-chunk, 16 accumulating
matmuls with the weight chunk stationary ([P(k),128c] x [P(k),8b] ->
[128c, 8b]) plus a K=1 bias matmul, all into one [P, CO, BL] PSUM bank,
then a single 140ns DVE cast to fp16.  No PE transposes and no per-chunk
PSUM->SBUF copies -- the old kernel's q phase kept DVE/ACT (the binding
engines at ~63us/rep) busy with copy work and plateaued at 67us/rep.

Scores are computed REPLICATED across all 128 partitions (the stationary
q column is free-dim-broadcast to 128 identical columns; moving cost is
unchanged), so reduce_max / Exp / reciprocal all produce per-partition
results directly and the two GpSimd partition_broadcasts per sample of
the old pipeline disappear -- freeing GpSimd for the q-phase copies and
the output store.

Cross-rep pipeline (emission order per rep r):
    loads(r) [wt + in2t + fcb on the ACT ring] -> phases 2-4 of rep
    r-1, with rep r's q-matmul chunk for c1-chunk b interleaved after
    sample b (filling the PE's idle gaps between score chains while
    rep r-1's x still streams) -> x loads(r) [sync ring, exclusive].
Thus rep r's weight load and whole q chain execute during rep r-1's
window, scores(r) can start the moment window r opens, and the sync
ring streams x back-to-back across reps (the output store rides the
GpSimd queue, so it never stalls the x stream the way the old kernel's
sync-ring store did).  (x tile slots rotate through a 16-deep pool;
emitting phases234(r-1) before x(r) keeps the pool's WAR tracking
exact.)

Per-sample phases 2-4 (software-pipelined: pool(b-1) after softmax(b)):
  2. scores: 16 M=128-replicated TensorE matmuls accumulating over the
     8 C1-chunks into one [P, 2, 392] PSUM tile.
  3. softmax: DVE negated reduce_max -> [P,1]; one ACT Exp (bias=-max,
     accum_out=sum) writing the fp16 attn row replicated [P, 784]; DVE
     reciprocal -> [P,1].
  4. pooling per C1-chunk, HW-A/B-tuned split: 6 chunks DVE
     tensor_tensor mult (2x fp16) + ACT Copy(scale=1/sum, accum_out);
     the last 2 chunks as one paired DVE mult [P,2,HW] + one axis-X
     paired reduce + a tiny normalize (measured cheaper than two 1x
     scalar_tensor_tensors; (7,1) and (5,3) splits measured 79/70us
     per rep vs 66 for this arrangement -- DVE and ACT are co-binding
     at ~8.2us/sample while the pure DMA stream is only 55us/rep).
"""

import numpy as np

import concourse.bacc as bacc
import concourse.mybir as mybir
import concourse.tile as tile
from concourse.bass_utils import run_bass_kernel_spmd

F32 = mybir.dt.float32
F16 = mybir.dt.float16

B, C1, C2, HW = 64, 1024, 2048, 784
NCORES = 8
BL = B // NCORES          # samples per core
P = 128                   # partitions
CO = C1 // P              # 8 c1 chunks
KC = C2 // P              # 16 c2 chunks
HH = HW // 2              # 392, half the pixels
XH = 2                    # x DMA split: halves of the c1-chunks per sample
COH = CO // XH            # c1-chunks per x half-tile
NTT = 6                   # pooling chunks: DVE tensor_tensor + ACT accum
NST = 2                   # pooling chunks: DVE scalar_tensor_tensor
SMBUFS = 4                # softmax/pooling small-tile pool depth
WMBUFS = 4                # wm (DVE mult output) rotation depth
WABUFS = 2                # wa (ACT accumulate elementwise out) depth
SPBUFS = 3                # scores PSUM tile depth (2 banks each)

_CACHE = {}


def _build(repeat=1):
    nc = bacc.Bacc(
        "TRN2", target_bir_lowering=False, debug=False, num_devices=NCORES
    )

    x = nc.dram_tensor("x", [BL, P, CO * HW], F16, kind="ExternalInput").ap()
    wt = nc.dram_tensor("wt", [C2, C1], F16, kind="ExternalInput").ap()
    in2t = nc.dram_tensor("in2t", [P, KC * BL], F16,
                          kind="ExternalInput").ap()
    fcb = nc.dram_tensor("fcb", [1, C1], F16, kind="ExternalInput").ap()
    out = nc.dram_tensor("out", [P, BL * CO], F32, kind="ExternalOutput").ap()
    with tile.TileContext(nc) as tc:
        _emit(tc, nc, x, wt, in2t, fcb, out, repeat=repeat)

    nc.compile()
    return nc


def _emit(tc, nc, x, wt, in2t, fcb, out, repeat=1):
    import contextlib

    ctx = contextlib.ExitStack()
    with ctx:
        const = ctx.enter_context(tc.tile_pool(name="const", bufs=1))
        wtp = ctx.enter_context(tc.tile_pool(name="wtp", bufs=2))
        xp = ctx.enter_context(tc.tile_pool(name="xp", bufs=BL * XH))
        sm = ctx.enter_context(tc.tile_pool(name="sm", bufs=SMBUFS))
        q_pp = ctx.enter_context(
            tc.tile_pool(name="q_pp", bufs=1, space="PSUM")
        )
        s_pp = ctx.enter_context(
            tc.tile_pool(name="s_pp", bufs=SPBUFS, space="PSUM")
        )
        wa_pp = ctx.enter_context(
            tc.tile_pool(name="wa_pp", bufs=1, space="PSUM")
        )

        ones_sb = const.tile([1, BL], F16, name="ones_sb", tag="ones_sb")
        nc.vector.memset(ones_sb[:], 1.0)

        xr = x.rearrange("b p (h c q) -> b p h c q", h=XH, c=COH)
        wtr = wt.rearrange("(k p) c -> p k c", p=P)

        prev = None
        for rep in range(repeat):
            # ---- loads for this rep (ACT HWDGE ring) ---------------------
            in2t_sb = wtp.tile([P, KC * BL], F16, name="in2t_sb",
                               tag="in2t_sb")
            nc.scalar.dma_start(out=in2t_sb[:], in_=in2t)
            fcb_sb = wtp.tile([1, C1], F16, name="fcb_sb", tag="fcb_sb")
            nc.scalar.dma_start(out=fcb_sb[:], in_=fcb)
            # wt rides the sync ring AHEAD of x, one 525KB slab per
            # c1-chunk, so interleaved q-matmul chunk co fires as soon as
            # its slab lands (~1.5us * (co+1) into the previous window)
            wt_sb = wtp.tile([P, KC, C1], F16, name="wt_sb", tag="wt_sb")
            for co in range(CO):
                cs = slice(co * P, (co + 1) * P)
                nc.sync.dma_start(out=wt_sb[:, :, cs], in_=wtr[:, :, cs])

            # ---- q chain for this rep: qT computed directly --------------
            # emitted as 8 per-c1-chunk closures, interleaved into the
            # previous rep's sample loop to fill PE idle gaps
            q_ps = q_pp.tile([P, CO, BL], F32, name="q_ps", tag="q_ps")
            qt_all = wtp.tile([P, CO, BL], F16, name="qt_all", tag="qt_all")

            def _mk_qmm(co, q_ps=q_ps, in2t_sb=in2t_sb, fcb_sb=fcb_sb,
                        wt_sb=wt_sb):
                def emit():
                    for k in range(KC):
                        nc.tensor.matmul(
                            q_ps[:, co, :],
                            wt_sb[:, k, co * P:(co + 1) * P],
                            in2t_sb[:, k * BL:(k + 1) * BL],
                            start=(k == 0),
                            stop=False,
                        )
                    nc.tensor.matmul(
                        q_ps[:, co, :],
                        fcb_sb[0:1, co * P:(co + 1) * P],
                        ones_sb[0:1, 0:BL],
                        start=False,
                        stop=True,
                    )
                return emit

            def _qcast(q_ps=q_ps, qt_all=qt_all):
                nc.vector.tensor_copy(qt_all[:], q_ps[:])

            qmm = [_mk_qmm(co) for co in range(CO)] + [_qcast]

            # ---- phases 2-4 of the PREVIOUS rep --------------------------
            if prev is not None:
                _emit_body(tc, nc, s_pp, sm, wtp, out, *prev, qmm=qmm,
                           wa_pp=wa_pp)
            else:
                for f in qmm:
                    f()

            # ---- x loads for this rep (sync ring, exclusive) -------------
            x_sb = []
            for b in range(BL):
                halves = []
                for h in range(XH):
                    t = xp.tile([P, COH, HW], F16, name="x_sb", tag="x_sb")
                    nc.sync.dma_start(out=t[:], in_=xr[b, :, h])
                    halves.append(t)
                x_sb.append(halves)

            prev = (x_sb, qt_all)

        _emit_body(tc, nc, s_pp, sm, wtp, out, *prev, qmm=[], wa_pp=wa_pp)


def _emit_body(tc, nc, s_pp, sm, wtp, out, x_sb, qt_all, qmm=(),
               wa_pp=None):
    # per-sample scores/softmax/pooling, software-pipelined, then store
    gall = wtp.tile([P, BL * CO], F32, name="gall", tag="gall")
    atiles = {}
    for b in range(BL):
        atiles[("s_ps", b)] = _emit_scores(nc, s_pp, x_sb, qt_all, b)
        if b < len(qmm):
            qmm[b]()       # next rep's q matmuls fill this PE idle gap
        _emit_softmax(nc, sm, b, atiles)
        if b >= 1:
            _emit_pool(nc, sm, x_sb, gall, b - 1, atiles, wa_pp)
        # reciprocal AFTER pool(b-1): it waits on the ACT Exp, and the
        # in-order DVE must not head-block the pooling work behind it
        _emit_recip(nc, sm, b, atiles)
    _emit_pool(nc, sm, x_sb, gall, BL - 1, atiles, wa_pp)
    for f in qmm[BL:]:
        f()
    # store from the GpSimd queue: it has no per-sample work, so a waiting
    # dma_start never blocks anything on the critical path
    nc.gpsimd.dma_start(out=out, in_=gall[:])


def _emit_scores(nc, s_pp, x_sb, qt_all, b):
    # 16 matmuls; the stationary q column is broadcast to 128 identical
    # columns so every output partition carries the same score row
    # [P, 2, 512] f32 = 4KB/partition = exactly 2 PSUM banks, so each
    # 392-wide half-slice stays inside its own bank
    s_ps = s_pp.tile([P, 2, 512], F32, name="s_ps", tag="s_ps")
    for co in range(CO):
        xt = x_sb[b][co // COH]
        stat = qt_all[:, co, b:b + 1].broadcast_to((P, P))
        for h in range(2):
            nc.tensor.matmul(
                s_ps[:, h, 0:HH],
                stat,
                xt[:, co % COH, h * HH:(h + 1) * HH],
                start=(co == 0),
                stop=(co == CO - 1),
            )
    return s_ps


def _emit_softmax(nc, sm, b, atiles):
    s_ps = atiles.pop(("s_ps", b))
    nm = sm.tile([P, 1], F32, name="nm", tag="nm")
    nc.vector.tensor_reduce(
        nm[:], s_ps[:, :, 0:HH], axis=mybir.AxisListType.XY,
        op=mybir.AluOpType.max, negate=True,
    )
    l = sm.tile([P, 1], F32, name="l", tag="l")
    a_sb = sm.tile([P, HW], F16, name="a_sb", tag="a_sb")
    nc.scalar.activation(
        a_sb.rearrange("p (h n) -> p h n", h=2),
        s_ps[:, :, 0:HH],
        mybir.ActivationFunctionType.Exp,
        bias=nm[:], accum_out=l[:],
    )
    atiles[("a", b)] = a_sb
    atiles[("l", b)] = l


def _emit_recip(nc, sm, b, atiles):
    l = atiles.pop(("l", b))
    r_bc = sm.tile([P, 1], F32, name="r_bc", tag="r_bc")
    nc.vector.reciprocal(r_bc[:], l[:])
    atiles[("r", b)] = r_bc


def _emit_pool(nc, sm, x_sb, gall, b, atiles, wa_pp):
    # 6 chunks DVE mult (2x fp16) + ACT accumulate w/ 1/sum scale (wa
    # rides the free 8th PSUM bank: cheaper ACT write init than SBUF);
    # last 2 chunks as ONE paired DVE mult + ONE axis-X paired reduce +
    # a tiny normalize (cheaper than two 1x scalar_tensor_tensors)
    a_sb = atiles.pop(("a", b))
    r_bc = atiles.pop(("r", b))

    def chunk(co):
        return x_sb[b][co // COH][:, co % COH, :]

    wa = sm.tile([P, HW], F16, name="wa", tag="wa", bufs=WABUFS)
    co = 0
    for _ in range(NTT):
        wm = sm.tile([P, HW], F16, name="wm", tag="wm", bufs=WMBUFS)
        nc.vector.tensor_tensor(
            out=wm[:], in0=chunk(co), in1=a_sb[:], op=mybir.AluOpType.mult
        )
        nc.scalar.activation(
            wa[:], wm[:], mybir.ActivationFunctionType.Copy,
            scale=r_bc[:],
            accum_out=gall[:, b * CO + co:b * CO + co + 1],
        )
        co += 1
    # co = 6, 7 live in x half 1, positions 2..4: one contiguous pair
    hx, cl = co // COH, co % COH
    wm2 = sm.tile([P, 2, HW], F16, name="wm2", tag="wm2", bufs=WMBUFS)
    a2 = a_sb[:].unsqueeze(1).broadcast_to((P, 2, HW))
    nc.vector.tensor_tensor(
        out=wm2[:], in0=x_sb[b][hx][:, cl:cl + 2, :], in1=a2,
        op=mybir.AluOpType.mult,
    )
    gu2 = sm.tile([P, 2], F32, name="gu2", tag="gu2")
    nc.vector.tensor_reduce(
        gu2[:], wm2[:], axis=mybir.AxisListType.X, op=mybir.AluOpType.add,
    )
    nc.vector.scalar_tensor_tensor(
        out=gall[:, b * CO + co:b * CO + co + 2], in0=gu2[:],
        scalar=r_bc[:], in1=gu2[:],
        op0=mybir.AluOpType.mult, op1=mybir.AluOpType.bypass,
    )


def _get_nc():
    if "nc" not in _CACHE:
        _CACHE["nc"] = _build()
    return _CACHE["nc"]


def _in_maps(input1, input2, fc_w, fc_b):
    input1 = np.asarray(input1, dtype=np.float32)
    input2 = np.asarray(input2, dtype=np.float32)
    fc_w = np.asarray(fc_w, dtype=np.float32)
    fc_b = np.asarray(fc_b, dtype=np.float32)

    wt = np.ascontiguousarray(fc_w.T.astype(np.float16))      # [C2, C1]
    fcb = np.ascontiguousarray(fc_b.reshape(1, C1).astype(np.float16))
    maps = []
    for i in range(NCORES):
        sl = slice(i * BL, (i + 1) * BL)
        # x[b, co*128+ci, q] -> [b, ci, co*HW+q]
        x_sh = np.ascontiguousarray(
            input1[sl]
            .reshape(BL, CO, P, HW)
            .transpose(0, 2, 1, 3)
            .reshape(BL, P, CO * HW)
            .astype(np.float16)
        )
        # in2t[p, k*BL + b] = input2[i*BL + b, k*128 + p]
        i2t = np.ascontiguousarray(
            input2[sl].T.reshape(KC, P, BL)
            .transpose(1, 0, 2)
            .reshape(P, KC * BL)
            .astype(np.float16)
        )
        maps.append({"x": x_sh, "wt": wt, "in2t": i2t, "fcb": fcb})
    return maps


def _assemble(results):
    outs = []
    for i in range(NCORES):
        arr = np.asarray(results[i]["out"])                 # [128, BL*CO]
        # arr[ci, b*CO + co] = g[b, co*128 + ci]
        outs.append(
            arr.reshape(P, BL, CO).transpose(1, 2, 0).reshape(BL, C1)
        )
    return np.ascontiguousarray(
        np.concatenate(outs, axis=0).astype(np.float32)
    )


def run(input1, input2, fc_w, fc_b, trace=False, **trace_kwargs):
    nc = _get_nc()
    res = run_bass_kernel_spmd(
        nc,
        _in_maps(input1, input2, fc_w, fc_b),
        core_ids=list(range(NCORES)),
        trace=trace,
        **trace_kwargs,
    )
    return _assemble(res.results), res


def kernel(input1, input2, fc_w, fc_b):
    out, _ = run(input1, input2, fc_w, fc_b)
    return out


# revision 4
# speedup vs baseline: 1.0278x; 1.0278x over previous
"""Trainium2 Bass kernel for the attention-pooling layer (fp16 pipeline),
data-parallel over batch with replicated fc_w, cross-rep software
pipelined.

Computation (per sample b):
    q = input2 @ fc_w.T + fc_b                      # [B, C1]
    scores[b, p] = <input1[b, :, p], q[b]>          # [B, HW]
    attn = softmax(scores, axis=1)
    out[b, c] = sum_p input1[b, c, p] * attn[b, p]  # [B, C1]

Sharding: x is data-parallel over batch (8 samples/core), fc_w
replicated -- measured A/B: an AllToAll/ReduceScatter q-exchange with
sharded fc_w costs ~12us/~90us of hard-serialized collective time per
rep on this stack, far more than the 3.7MB of HBM it saves (the kernel
is compute-bound at ~63us/rep, DMA is 47.9us).  q for the core's own 8
samples is computed DIRECTLY TRANSPOSED: per c1-chunk, 16 accumulating
matmuls with the weight chunk stationary ([P(k),128c] x [P(k),8b] ->
[128c, 8b]) plus a K=1 bias matmul, all into one [P, CO, BL] PSUM bank,
then a single 140ns DVE cast to fp16.  No PE transposes and no per-chunk
PSUM->SBUF copies -- the old kernel's q phase kept DVE/ACT (the binding
engines at ~63us/rep) busy with copy work and plateaued at 67us/rep.

Scores are computed REPLICATED across all 128 partitions (the stationary
q column is free-dim-broadcast to 128 identical columns; moving cost is
unchanged), so reduce_max / Exp / reciprocal all produce per-partition
results directly and the two GpSimd partition_broadcasts per sample of
the old pipeline disappear -- freeing GpSimd for the q-phase copies and
the output store.

Cross-rep pipeline (emission order per rep r):
    loads(r) [wt + in2t + fcb on the ACT ring] -> phases 2-4 of rep
    r-1, with rep r's q-matmul chunk for c1-chunk b interleaved after
    sample b (filling the PE's idle gaps between score chains while
    rep r-1's x still streams) -> x loads(r) [sync ring, exclusive].
Thus rep r's weight load and whole q chain execute during rep r-1's
window, scores(r) can start the moment window r opens, and the sync
ring streams x back-to-back across reps (the output store rides the
GpSimd queue, so it never stalls the x stream the way the old kernel's
sync-ring store did).  (x tile slots rotate through a 16-deep pool;
emitting phases234(r-1) before x(r) keeps the pool's WAR tracking
exact.)

Per-sample phases 2-4 (software-pipelined: pool(b-1) after softmax(b)):
  2. scores: 16 M=128-replicated TensorE matmuls accumulating over the
     8 C1-chunks into one [P, 2, 392] PSUM tile.
  3. softmax: DVE negated reduce_max -> [P,1]; one ACT Exp (bias=-max,
     accum_out=sum) writing the fp16 attn row replicated [P, 784]; DVE
     reciprocal -> [P,1].
  4. pooling per C1-chunk, HW-A/B-tuned split: 6 chunks DVE
     tensor_tensor mult (2x fp16) + ACT Copy(scale=1/sum, accum_out);
     the last 3 chunks as one grouped DVE mult [P,3,HW] + one axis-X
     grouped reduce + a tiny normalize.  Measured ladder: (7,1)=79,
     (5,3)=70, (6,2)+stt=66.1, (6 copies + paired-2)=65.9, this
     (5 copies + grouped-3)=64.5us/rep -- DVE and ACT are co-binding
     near ~8us/sample while the pure DMA stream is 55us/rep.
     (tensor_scalar's 4x_2p accum path does not lower through walrus;
     tensor_tensor tops out at 2x_1p; tensor_reduce has no fast mode.)
"""

import numpy as np

import concourse.bacc as bacc
import concourse.mybir as mybir
import concourse.tile as tile
from concourse.bass_utils import run_bass_kernel_spmd

F32 = mybir.dt.float32
F16 = mybir.dt.float16

B, C1, C2, HW = 64, 1024, 2048, 784
NCORES = 8
BL = B // NCORES          # samples per core
P = 128                   # partitions
CO = C1 // P              # 8 c1 chunks
KC = C2 // P              # 16 c2 chunks
HH = HW // 2              # 392, half the pixels
XH = 2                    # x DMA split: halves of the c1-chunks per sample
COH = CO // XH            # c1-chunks per x half-tile
NTT = 5                   # pooling chunks: DVE tensor_tensor + ACT accum
NST = 2                   # pooling chunks: DVE scalar_tensor_tensor
SMBUFS = 4                # softmax/pooling small-tile pool depth
WMBUFS = 4                # wm (DVE mult output) rotation depth
WABUFS = 2                # wa (ACT accumulate elementwise out) depth
SPBUFS = 3                # scores PSUM tile depth (2 banks each)

_CACHE = {}


def _build(repeat=1):
    nc = bacc.Bacc(
        "TRN2", target_bir_lowering=False, debug=False, num_devices=NCORES
    )

    x = nc.dram_tensor("x", [BL, P, CO * HW], F16, kind="ExternalInput").ap()
    wt = nc.dram_tensor("wt", [C2, C1], F16, kind="ExternalInput").ap()
    in2t = nc.dram_tensor("in2t", [P, KC * BL], F16,
                          kind="ExternalInput").ap()
    fcb = nc.dram_tensor("fcb", [1, C1], F16, kind="ExternalInput").ap()
    out = nc.dram_tensor("out", [P, BL * CO], F32, kind="ExternalOutput").ap()
    with tile.TileContext(nc) as tc:
        _emit(tc, nc, x, wt, in2t, fcb, out, repeat=repeat)

    nc.compile()
    return nc


def _emit(tc, nc, x, wt, in2t, fcb, out, repeat=1):
    import contextlib

    ctx = contextlib.ExitStack()
    with ctx:
        const = ctx.enter_context(tc.tile_pool(name="const", bufs=1))
        wtp = ctx.enter_context(tc.tile_pool(name="wtp", bufs=2))
        xp = ctx.enter_context(tc.tile_pool(name="xp", bufs=BL * XH))
        sm = ctx.enter_context(tc.tile_pool(name="sm", bufs=SMBUFS))
        q_pp = ctx.enter_context(
            tc.tile_pool(name="q_pp", bufs=1, space="PSUM")
        )
        s_pp = ctx.enter_context(
            tc.tile_pool(name="s_pp", bufs=SPBUFS, space="PSUM")
        )
        wa_pp = ctx.enter_context(
            tc.tile_pool(name="wa_pp", bufs=1, space="PSUM")
        )

        ones_sb = const.tile([1, BL], F16, name="ones_sb", tag="ones_sb")
        nc.vector.memset(ones_sb[:], 1.0)

        xr = x.rearrange("b p (h c q) -> b p h c q", h=XH, c=COH)
        wtr = wt.rearrange("(k p) c -> p k c", p=P)

        prev = None
        for rep in range(repeat):
            # ---- loads for this rep (ACT HWDGE ring) ---------------------
            in2t_sb = wtp.tile([P, KC * BL], F16, name="in2t_sb",
                               tag="in2t_sb")
            nc.scalar.dma_start(out=in2t_sb[:], in_=in2t)
            fcb_sb = wtp.tile([1, C1], F16, name="fcb_sb", tag="fcb_sb")
            nc.scalar.dma_start(out=fcb_sb[:], in_=fcb)
            # wt rides the sync ring AHEAD of x, one 525KB slab per
            # c1-chunk, so interleaved q-matmul chunk co fires as soon as
            # its slab lands (~1.5us * (co+1) into the previous window)
            wt_sb = wtp.tile([P, KC, C1], F16, name="wt_sb", tag="wt_sb")
            for co in range(CO):
                cs = slice(co * P, (co + 1) * P)
                nc.sync.dma_start(out=wt_sb[:, :, cs], in_=wtr[:, :, cs])

            # ---- q chain for this rep: qT computed directly --------------
            # emitted as 8 per-c1-chunk closures, interleaved into the
            # previous rep's sample loop to fill PE idle gaps
            q_ps = q_pp.tile([P, CO, BL], F32, name="q_ps", tag="q_ps")
            qt_all = wtp.tile([P, CO, BL], F16, name="qt_all", tag="qt_all")

            def _mk_qmm(co, q_ps=q_ps, in2t_sb=in2t_sb, fcb_sb=fcb_sb,
                        wt_sb=wt_sb):
                def emit():
                    for k in range(KC):
                        nc.tensor.matmul(
                            q_ps[:, co, :],
                            wt_sb[:, k, co * P:(co + 1) * P],
                            in2t_sb[:, k * BL:(k + 1) * BL],
                            start=(k == 0),
                            stop=False,
                        )
                    nc.tensor.matmul(
                        q_ps[:, co, :],
                        fcb_sb[0:1, co * P:(co + 1) * P],
                        ones_sb[0:1, 0:BL],
                        start=False,
                        stop=True,
                    )
                return emit

            def _qcast(q_ps=q_ps, qt_all=qt_all):
                nc.vector.tensor_copy(qt_all[:], q_ps[:])

            qmm = [_mk_qmm(co) for co in range(CO)] + [_qcast]

            # ---- phases 2-4 of the PREVIOUS rep --------------------------
            if prev is not None:
                _emit_body(tc, nc, s_pp, sm, wtp, out, *prev, qmm=qmm,
                           wa_pp=wa_pp)
            else:
                for f in qmm:
                    f()

            # ---- x loads for this rep (sync ring, exclusive) -------------
            x_sb = []
            for b in range(BL):
                halves = []
                for h in range(XH):
                    t = xp.tile([P, COH, HW], F16, name="x_sb", tag="x_sb")
                    nc.sync.dma_start(out=t[:], in_=xr[b, :, h])
                    halves.append(t)
                x_sb.append(halves)

            prev = (x_sb, qt_all)

        _emit_body(tc, nc, s_pp, sm, wtp, out, *prev, qmm=[], wa_pp=wa_pp)


def _emit_body(tc, nc, s_pp, sm, wtp, out, x_sb, qt_all, qmm=(),
               wa_pp=None):
    # per-sample scores/softmax/pooling, software-pipelined, then store
    gall = wtp.tile([P, BL * CO], F32, name="gall", tag="gall")
    atiles = {}
    for b in range(BL):
        atiles[("s_ps", b)] = _emit_scores(nc, s_pp, x_sb, qt_all, b)
        if b < len(qmm):
            qmm[b]()       # next rep's q matmuls fill this PE idle gap
        _emit_softmax(nc, sm, b, atiles)
        if b >= 1:
            _emit_pool(nc, sm, x_sb, gall, b - 1, atiles, wa_pp)
        # reciprocal AFTER pool(b-1): it waits on the ACT Exp, and the
        # in-order DVE must not head-block the pooling work behind it
        _emit_recip(nc, sm, b, atiles)
    _emit_pool(nc, sm, x_sb, gall, BL - 1, atiles, wa_pp)
    for f in qmm[BL:]:
        f()
    # store from the GpSimd queue: it has no per-sample work, so a waiting
    # dma_start never blocks anything on the critical path
    nc.gpsimd.dma_start(out=out, in_=gall[:])


def _emit_scores(nc, s_pp, x_sb, qt_all, b):
    # 16 matmuls; the stationary q column is broadcast to 128 identical
    # columns so every output partition carries the same score row
    # [P, 2, 512] f32 = 4KB/partition = exactly 2 PSUM banks, so each
    # 392-wide half-slice stays inside its own bank
    s_ps = s_pp.tile([P, 2, 512], F32, name="s_ps", tag="s_ps")
    for co in range(CO):
        xt = x_sb[b][co // COH]
        stat = qt_all[:, co, b:b + 1].broadcast_to((P, P))
        for h in range(2):
            nc.tensor.matmul(
                s_ps[:, h, 0:HH],
                stat,
                xt[:, co % COH, h * HH:(h + 1) * HH],
                start=(co == 0),
                stop=(co == CO - 1),
            )
    return s_ps


def _emit_softmax(nc, sm, b, atiles):
    s_ps = atiles.pop(("s_ps", b))
    nm = sm.tile([P, 1], F32, name="nm", tag="nm")
    nc.vector.tensor_reduce(
        nm[:], s_ps[:, :, 0:HH], axis=mybir.AxisListType.XY,
        op=mybir.AluOpType.max, negate=True,
    )
    l = sm.tile([P, 1], F32, name="l", tag="l")
    a_sb = sm.tile([P, HW], F16, name="a_sb", tag="a_sb")
    nc.scalar.activation(
        a_sb.rearrange("p (h n) -> p h n", h=2),
        s_ps[:, :, 0:HH],
        mybir.ActivationFunctionType.Exp,
        bias=nm[:], accum_out=l[:],
    )
    atiles[("a", b)] = a_sb
    atiles[("l", b)] = l


def _emit_recip(nc, sm, b, atiles):
    l = atiles.pop(("l", b))
    r_bc = sm.tile([P, 1], F32, name="r_bc", tag="r_bc")
    nc.vector.reciprocal(r_bc[:], l[:])
    atiles[("r", b)] = r_bc


def _emit_pool(nc, sm, x_sb, gall, b, atiles, wa_pp):
    # 6 chunks DVE mult (2x fp16) + ACT accumulate w/ 1/sum scale (wa
    # rides the free 8th PSUM bank: cheaper ACT write init than SBUF);
    # last 2 chunks as ONE paired DVE mult + ONE axis-X paired reduce +
    # a tiny normalize (cheaper than two 1x scalar_tensor_tensors)
    a_sb = atiles.pop(("a", b))
    r_bc = atiles.pop(("r", b))

    def chunk(co):
        return x_sb[b][co // COH][:, co % COH, :]

    NG = CO - NTT          # trailing chunks grouped into one DVE op pair
    wa = sm.tile([P, HW], F16, name="wa", tag="wa", bufs=WABUFS)
    co = 0
    for _ in range(NTT):
        wm = sm.tile([P, HW], F16, name="wm", tag="wm", bufs=WMBUFS)
        nc.vector.tensor_tensor(
            out=wm[:], in0=chunk(co), in1=a_sb[:], op=mybir.AluOpType.mult
        )
        nc.scalar.activation(
            wa[:], wm[:], mybir.ActivationFunctionType.Copy,
            scale=r_bc[:],
            accum_out=gall[:, b * CO + co:b * CO + co + 1],
        )
        co += 1
    # trailing NG chunks all live contiguously in x half 1: ONE grouped
    # DVE mult [P, NG, HW] + ONE axis-X grouped reduce + a tiny normalize
    hx, cl = co // COH, co % COH
    wm2 = sm.tile([P, NG, HW], F16, name="wm2", tag="wm2", bufs=WMBUFS)
    a2 = a_sb[:].unsqueeze(1).broadcast_to((P, NG, HW))
    nc.vector.tensor_tensor(
        out=wm2[:], in0=x_sb[b][hx][:, cl:cl + NG, :], in1=a2,
        op=mybir.AluOpType.mult,
    )
    gu2 = sm.tile([P, NG], F32, name="gu2", tag="gu2")
    nc.vector.tensor_reduce(
        gu2[:], wm2[:], axis=mybir.AxisListType.X, op=mybir.AluOpType.add,
    )
    nc.vector.scalar_tensor_tensor(
        out=gall[:, b * CO + co:b * CO + co + NG], in0=gu2[:],
        scalar=r_bc[:], in1=gu2[:],
        op0=mybir.AluOpType.mult, op1=mybir.AluOpType.bypass,
    )


def _get_nc():
    if "nc" not in _CACHE:
        _CACHE["nc"] = _build()
    return _CACHE["nc"]


def _in_maps(input1, input2, fc_w, fc_b):
    input1 = np.asarray(input1, dtype=np.float32)
    input2 = np.asarray(input2, dtype=np.float32)
    fc_w = np.asarray(fc_w, dtype=np.float32)
    fc_b = np.asarray(fc_b, dtype=np.float32)

    wt = np.ascontiguousarray(fc_w.T.astype(np.float16))      # [C2, C1]
    fcb = np.ascontiguousarray(fc_b.reshape(1, C1).astype(np.float16))
    maps = []
    for i in range(NCORES):
        sl = slice(i * BL, (i + 1) * BL)
        # x[b, co*128+ci, q] -> [b, ci, co*HW+q]
        x_sh = np.ascontiguousarray(
            input1[sl]
            .reshape(BL, CO, P, HW)
            .transpose(0, 2, 1, 3)
            .reshape(BL, P, CO * HW)
            .astype(np.float16)
        )
        # in2t[p, k*BL + b] = input2[i*BL + b, k*128 + p]
        i2t = np.ascontiguousarray(
            input2[sl].T.reshape(KC, P, BL)
            .transpose(1, 0, 2)
            .reshape(P, KC * BL)
            .astype(np.float16)
        )
        maps.append({"x": x_sh, "wt": wt, "in2t": i2t, "fcb": fcb})
    return maps


def _assemble(results):
    outs = []
    for i in range(NCORES):
        arr = np.asarray(results[i]["out"])                 # [128, BL*CO]
        # arr[ci, b*CO + co] = g[b, co*128 + ci]
        outs.append(
            arr.reshape(P, BL, CO).transpose(1, 2, 0).reshape(BL, C1)
        )
    return np.ascontiguousarray(
        np.concatenate(outs, axis=0).astype(np.float32)
    )


def run(input1, input2, fc_w, fc_b, trace=False, **trace_kwargs):
    nc = _get_nc()
    res = run_bass_kernel_spmd(
        nc,
        _in_maps(input1, input2, fc_w, fc_b),
        core_ids=list(range(NCORES)),
        trace=trace,
        **trace_kwargs,
    )
    return _assemble(res.results), res


def kernel(input1, input2, fc_w, fc_b):
    out, _ = run(input1, input2, fc_w, fc_b)
    return out


# revision 5
# speedup vs baseline: 1.0694x; 1.0405x over previous
"""Trainium2 Bass kernel for the attention-pooling layer (fp16 pipeline),
data-parallel over batch with replicated fc_w, cross-rep software
pipelined.

Computation (per sample b):
    q = input2 @ fc_w.T + fc_b                      # [B, C1]
    scores[b, p] = <input1[b, :, p], q[b]>          # [B, HW]
    attn = softmax(scores, axis=1)
    out[b, c] = sum_p input1[b, c, p] * attn[b, p]  # [B, C1]

Sharding: x is data-parallel over batch (8 samples/core), fc_w
replicated -- measured A/B: an AllToAll/ReduceScatter q-exchange with
sharded fc_w costs ~12us/~90us of hard-serialized collective time per
rep on this stack, far more than the 3.7MB of HBM it saves (the kernel
is compute-bound at ~63us/rep, DMA is 47.9us).  q for the core's own 8
samples is computed DIRECTLY TRANSPOSED: per c1-chunk, 16 accumulating
matmuls with the weight chunk stationary ([P(k),128c] x [P(k),8b] ->
[128c, 8b]) plus a K=1 bias matmul, all into one [P, CO, BL] PSUM bank,
then a single 140ns DVE cast to fp16.  No PE transposes and no per-chunk
PSUM->SBUF copies -- the old kernel's q phase kept DVE/ACT (the binding
engines at ~63us/rep) busy with copy work and plateaued at 67us/rep.

Scores are computed REPLICATED across all 128 partitions (the stationary
q column is free-dim-broadcast to 128 identical columns; moving cost is
unchanged), so reduce_max / Exp / reciprocal all produce per-partition
results directly and the two GpSimd partition_broadcasts per sample of
the old pipeline disappear -- freeing GpSimd for the q-phase copies and
the output store.

Cross-rep pipeline (emission order per rep r):
    loads(r) [wt + in2t + fcb on the ACT ring] -> phases 2-4 of rep
    r-1, with rep r's q-matmul chunk for c1-chunk b interleaved after
    sample b (filling the PE's idle gaps between score chains while
    rep r-1's x still streams) -> x loads(r) [sync ring, exclusive].
Thus rep r's weight load and whole q chain execute during rep r-1's
window, scores(r) can start the moment window r opens, and the sync
ring streams x back-to-back across reps (the output store rides the
GpSimd queue, so it never stalls the x stream the way the old kernel's
sync-ring store did).  (x tile slots rotate through a 16-deep pool;
emitting phases234(r-1) before x(r) keeps the pool's WAR tracking
exact.)

Per-sample phases 2-4 (software-pipelined: pool(b-1) after softmax(b)):
  2. scores: 16 M=128-replicated TensorE matmuls accumulating over the
     8 C1-chunks into one [P, 2, 392] PSUM tile.
  3. softmax: DVE negated reduce_max -> [P,1]; one ACT Exp (bias=-max,
     accum_out=sum) writing the fp16 attn row replicated [P, 784]; DVE
     reciprocal -> [P,1].
  4. pooling per C1-chunk, HW-A/B-tuned split: 6 chunks DVE
     tensor_tensor mult (2x fp16) + ACT Copy(scale=1/sum, accum_out);
     the last 2 chunks as one paired DVE mult [P,2,HW] + one axis-X
     paired reduce + a tiny normalize (measured cheaper than two 1x
     scalar_tensor_tensors; (7,1) and (5,3) splits measured 79/70us
     per rep vs 66 for this arrangement -- DVE and ACT are co-binding
     at ~8.2us/sample while the pure DMA stream is only 55us/rep).
"""

import numpy as np

import concourse.bacc as bacc
import concourse.mybir as mybir
import concourse.tile as tile
from concourse.bass_utils import run_bass_kernel_spmd

F32 = mybir.dt.float32
F16 = mybir.dt.float16

B, C1, C2, HW = 64, 1024, 2048, 784
NCORES = 8
BL = B // NCORES          # samples per core
P = 128                   # partitions
CO = C1 // P              # 8 c1 chunks
KC = C2 // P              # 16 c2 chunks
HH = HW // 2              # 392, half the pixels
XH = 2                    # x DMA split: halves of the c1-chunks per sample
COH = CO // XH            # c1-chunks per x half-tile
NTT = 5                   # pooling chunks: DVE tensor_tensor + ACT accum
NST = 2                   # pooling chunks: DVE scalar_tensor_tensor
SMBUFS = 4                # softmax/pooling small-tile pool depth
WMBUFS = 4                # wm (DVE mult output) rotation depth
WABUFS = 2                # wa (ACT accumulate elementwise out) depth
SPBUFS = 3                # scores PSUM tile depth (2 banks each)

_CACHE = {}


def _build(repeat=1):
    nc = bacc.Bacc(
        "TRN2", target_bir_lowering=False, debug=False, num_devices=NCORES
    )

    x = nc.dram_tensor("x", [BL, P, CO * HW], F16, kind="ExternalInput").ap()
    wt = nc.dram_tensor("wt", [C2, C1], F16, kind="ExternalInput").ap()
    in2t = nc.dram_tensor("in2t", [P, KC * BL], F16,
                          kind="ExternalInput").ap()
    fcb = nc.dram_tensor("fcb", [1, C1], F16, kind="ExternalInput").ap()
    out = nc.dram_tensor("out", [P, BL * CO], F32, kind="ExternalOutput").ap()
    with tile.TileContext(nc) as tc:
        _emit(tc, nc, x, wt, in2t, fcb, out, repeat=repeat)

    nc.compile()
    return nc


def _emit(tc, nc, x, wt, in2t, fcb, out, repeat=1):
    import contextlib

    ctx = contextlib.ExitStack()
    with ctx:
        const = ctx.enter_context(tc.tile_pool(name="const", bufs=1))
        wtp = ctx.enter_context(tc.tile_pool(name="wtp", bufs=2))
        xp = ctx.enter_context(tc.tile_pool(name="xp", bufs=BL * XH))
        sm = ctx.enter_context(tc.tile_pool(name="sm", bufs=SMBUFS))
        q_pp = ctx.enter_context(
            tc.tile_pool(name="q_pp", bufs=1, space="PSUM")
        )
        s_pp = ctx.enter_context(
            tc.tile_pool(name="s_pp", bufs=SPBUFS, space="PSUM")
        )
        wa_pp = ctx.enter_context(
            tc.tile_pool(name="wa_pp", bufs=1, space="PSUM")
        )

        ones_sb = const.tile([1, BL], F16, name="ones_sb", tag="ones_sb")
        nc.vector.memset(ones_sb[:], 1.0)

        xr = x.rearrange("b p (h c q) -> b p h c q", h=XH, c=COH)
        wtr = wt.rearrange("(k p) c -> p k c", p=P)

        prev = None
        for rep in range(repeat):
            # ---- loads for this rep (ACT HWDGE ring) ---------------------
            in2t_sb = wtp.tile([P, KC * BL], F16, name="in2t_sb",
                               tag="in2t_sb")
            nc.scalar.dma_start(out=in2t_sb[:], in_=in2t)
            fcb_sb = wtp.tile([1, C1], F16, name="fcb_sb", tag="fcb_sb")
            nc.scalar.dma_start(out=fcb_sb[:], in_=fcb)
            # wt rides the sync ring AHEAD of x, one 525KB slab per
            # c1-chunk, so interleaved q-matmul chunk co fires as soon as
            # its slab lands (~1.5us * (co+1) into the previous window)
            wt_sb = wtp.tile([P, KC, C1], F16, name="wt_sb", tag="wt_sb")
            for co in range(CO):
                cs = slice(co * P, (co + 1) * P)
                nc.sync.dma_start(out=wt_sb[:, :, cs], in_=wtr[:, :, cs])

            # ---- q chain for this rep: qT computed directly --------------
            # emitted as 8 per-c1-chunk closures, interleaved into the
            # previous rep's sample loop to fill PE idle gaps
            q_ps = q_pp.tile([P, CO, BL], F32, name="q_ps", tag="q_ps")
            qt_all = wtp.tile([P, CO, BL], F16, name="qt_all", tag="qt_all")

            def _mk_qmm(co, q_ps=q_ps, in2t_sb=in2t_sb, fcb_sb=fcb_sb,
                        wt_sb=wt_sb):
                def emit():
                    for k in range(KC):
                        nc.tensor.matmul(
                            q_ps[:, co, :],
                            wt_sb[:, k, co * P:(co + 1) * P],
                            in2t_sb[:, k * BL:(k + 1) * BL],
                            start=(k == 0),
                            stop=False,
                        )
                    nc.tensor.matmul(
                        q_ps[:, co, :],
                        fcb_sb[0:1, co * P:(co + 1) * P],
                        ones_sb[0:1, 0:BL],
                        start=False,
                        stop=True,
                    )
                return emit

            def _qcast(q_ps=q_ps, qt_all=qt_all):
                nc.vector.tensor_copy(qt_all[:], q_ps[:])

            qmm = [_mk_qmm(co) for co in range(CO)] + [_qcast]

            # ---- phases 2-4 of the PREVIOUS rep --------------------------
            if prev is not None:
                _emit_body(tc, nc, s_pp, sm, wtp, out, *prev, qmm=qmm,
                           wa_pp=wa_pp)
            else:
                for f in qmm:
                    f()

            # ---- x loads for this rep (sync ring, exclusive) -------------
            x_sb = []
            for b in range(BL):
                halves = []
                for h in range(XH):
                    t = xp.tile([P, COH, HW], F16, name="x_sb", tag="x_sb")
                    nc.sync.dma_start(out=t[:], in_=xr[b, :, h])
                    halves.append(t)
                x_sb.append(halves)

            prev = (x_sb, qt_all)

        _emit_body(tc, nc, s_pp, sm, wtp, out, *prev, qmm=[], wa_pp=wa_pp)


def _emit_body(tc, nc, s_pp, sm, wtp, out, x_sb, qt_all, qmm=(),
               wa_pp=None):
    # per-sample scores/softmax/pooling, software-pipelined, then store
    gall = wtp.tile([P, BL * CO], F32, name="gall", tag="gall")
    atiles = {}
    for b in range(BL):
        atiles[("s_ps", b)] = _emit_scores(nc, s_pp, x_sb, qt_all, b)
        if b < len(qmm):
            qmm[b]()       # next rep's q matmuls fill this PE idle gap
        _emit_softmax(nc, sm, b, atiles)
        # grouped mults one sample back; ACT accumulation two samples
        # back, so the coarse [P,4,HW] group products are always a full
        # cycle old when the ACT Copies read them (no phasing stall)
        if b >= 1:
            _emit_pool_mult(nc, sm, x_sb, gall, b - 1, atiles)
        if b >= 2:
            _emit_pool_accum(nc, sm, gall, b - 2, atiles)
        # reciprocal last: it waits on the ACT Exp, and the in-order DVE
        # must not head-block pooling work behind it
        _emit_recip(nc, sm, b, atiles)
    _emit_pool_mult(nc, sm, x_sb, gall, BL - 1, atiles)
    _emit_pool_accum(nc, sm, gall, BL - 2, atiles)
    _emit_pool_accum(nc, sm, gall, BL - 1, atiles)
    for f in qmm[BL:]:
        f()
    # store from the GpSimd queue: it has no per-sample work, so a waiting
    # dma_start never blocks anything on the critical path
    nc.gpsimd.dma_start(out=out, in_=gall[:])


def _emit_scores(nc, s_pp, x_sb, qt_all, b):
    # 16 matmuls; the stationary q column is broadcast to 128 identical
    # columns so every output partition carries the same score row
    # [P, 2, 512] f32 = 4KB/partition = exactly 2 PSUM banks, so each
    # 392-wide half-slice stays inside its own bank
    s_ps = s_pp.tile([P, 2, 512], F32, name="s_ps", tag="s_ps")
    for co in range(CO):
        xt = x_sb[b][co // COH]
        stat = qt_all[:, co, b:b + 1].broadcast_to((P, P))
        for h in range(2):
            nc.tensor.matmul(
                s_ps[:, h, 0:HH],
                stat,
                xt[:, co % COH, h * HH:(h + 1) * HH],
                start=(co == 0),
                stop=(co == CO - 1),
            )
    return s_ps


def _emit_softmax(nc, sm, b, atiles):
    s_ps = atiles.pop(("s_ps", b))
    nm = sm.tile([P, 1], F32, name="nm", tag="nm")
    nc.vector.tensor_reduce(
        nm[:], s_ps[:, :, 0:HH], axis=mybir.AxisListType.XY,
        op=mybir.AluOpType.max, negate=True,
    )
    l = sm.tile([P, 1], F32, name="l", tag="l")
    a_sb = sm.tile([P, HW], F16, name="a_sb", tag="a_sb")
    nc.scalar.activation(
        a_sb.rearrange("p (h n) -> p h n", h=2),
        s_ps[:, :, 0:HH],
        mybir.ActivationFunctionType.Exp,
        bias=nm[:], accum_out=l[:],
    )
    atiles[("a", b)] = a_sb
    atiles[("l", b)] = l


def _emit_recip(nc, sm, b, atiles):
    l = atiles.pop(("l", b))
    r_bc = sm.tile([P, 1], F32, name="r_bc", tag="r_bc")
    nc.vector.reciprocal(r_bc[:], l[:])
    atiles[("r", b)] = r_bc


def _emit_pool_mult(nc, sm, x_sb, gall, b, atiles):
    # one grouped DVE mult per x half (2x fp16, [P, COH, HW]); the last
    # NG chunks also get their axis-X grouped reduce here (unnormalized)
    a_sb = atiles.pop(("a", b))
    wm4s = []
    for h in range(XH):
        wm4 = sm.tile([P, COH, HW], F16, name="wm4", tag="wm4", bufs=3)
        a4 = a_sb[:].unsqueeze(1).broadcast_to((P, COH, HW))
        nc.vector.tensor_tensor(
            out=wm4[:], in0=x_sb[b][h][:], in1=a4,
            op=mybir.AluOpType.mult,
        )
        wm4s.append(wm4)
    NG = CO - NTT
    gu = sm.tile([P, NG], F32, name="gu", tag="gu", bufs=3)
    nc.vector.tensor_reduce(
        gu[:], wm4s[1][:, COH - NG:COH, :], axis=mybir.AxisListType.X,
        op=mybir.AluOpType.add,
    )
    atiles[("wm4", b)] = wm4s
    atiles[("gu", b)] = gu


def _emit_pool_accum(nc, sm, gall, b, atiles):
    # NTT chunks: ACT Copy(scale=1/sum, accum_out) reading group-product
    # slices; the NG reduced chunks: one ACT Copy normalize of gu
    wm4s = atiles.pop(("wm4", b))
    gu = atiles.pop(("gu", b))
    r_bc = atiles.pop(("r", b))
    wa = sm.tile([P, HW], F16, name="wa", tag="wa", bufs=WABUFS)
    for co in range(NTT):
        nc.scalar.activation(
            wa[:], wm4s[co // COH][:, co % COH, :],
            mybir.ActivationFunctionType.Copy,
            scale=r_bc[:],
            accum_out=gall[:, b * CO + co:b * CO + co + 1],
        )
    NG = CO - NTT
    nc.scalar.activation(
        gall[:, b * CO + NTT:b * CO + CO], gu[:],
        mybir.ActivationFunctionType.Copy,
        scale=r_bc[:],
    )


def _get_nc():
    if "nc" not in _CACHE:
        _CACHE["nc"] = _build()
    return _CACHE["nc"]


def _in_maps(input1, input2, fc_w, fc_b):
    input1 = np.asarray(input1, dtype=np.float32)
    input2 = np.asarray(input2, dtype=np.float32)
    fc_w = np.asarray(fc_w, dtype=np.float32)
    fc_b = np.asarray(fc_b, dtype=np.float32)

    wt = np.ascontiguousarray(fc_w.T.astype(np.float16))      # [C2, C1]
    fcb = np.ascontiguousarray(fc_b.reshape(1, C1).astype(np.float16))
    maps = []
    for i in range(NCORES):
        sl = slice(i * BL, (i + 1) * BL)
        # x[b, co*128+ci, q] -> [b, ci, co*HW+q]
        x_sh = np.ascontiguousarray(
            input1[sl]
            .reshape(BL, CO, P, HW)
            .transpose(0, 2, 1, 3)
            .reshape(BL, P, CO * HW)
            .astype(np.float16)
        )
        # in2t[p, k*BL + b] = input2[i*BL + b, k*128 + p]
        i2t = np.ascontiguousarray(
            input2[sl].T.reshape(KC, P, BL)
            .transpose(1, 0, 2)
            .reshape(P, KC * BL)
            .astype(np.float16)
        )
        maps.append({"x": x_sh, "wt": wt, "in2t": i2t, "fcb": fcb})
    return maps


def _assemble(results):
    outs = []
    for i in range(NCORES):
        arr = np.asarray(results[i]["out"])                 # [128, BL*CO]
        # arr[ci, b*CO + co] = g[b, co*128 + ci]
        outs.append(
            arr.reshape(P, BL, CO).transpose(1, 2, 0).reshape(BL, C1)
        )
    return np.ascontiguousarray(
        np.concatenate(outs, axis=0).astype(np.float32)
    )


def run(input1, input2, fc_w, fc_b, trace=False, **trace_kwargs):
    nc = _get_nc()
    res = run_bass_kernel_spmd(
        nc,
        _in_maps(input1, input2, fc_w, fc_b),
        core_ids=list(range(NCORES)),
        trace=trace,
        **trace_kwargs,
    )
    return _assemble(res.results), res


def kernel(input1, input2, fc_w, fc_b):
    out, _ = run(input1, input2, fc_w, fc_b)
    return out


# revision 6
# speedup vs baseline: 1.0818x; 1.0116x over previous
"""Trainium2 Bass kernel for the attention-pooling layer (fp16 pipeline),
data-parallel over batch with replicated fc_w, cross-rep software
pipelined.

Computation (per sample b):
    q = input2 @ fc_w.T + fc_b                      # [B, C1]
    scores[b, p] = <input1[b, :, p], q[b]>          # [B, HW]
    attn = softmax(scores, axis=1)
    out[b, c] = sum_p input1[b, c, p] * attn[b, p]  # [B, C1]

Sharding: x is data-parallel over batch (8 samples/core), fc_w
replicated -- measured A/B: an AllToAll/ReduceScatter q-exchange with
sharded fc_w costs ~12us/~90us of hard-serialized collective time per
rep on this stack, far more than the 3.7MB of HBM it saves (the kernel
is compute-bound at ~63us/rep, DMA is 47.9us).  q for the core's own 8
samples is computed DIRECTLY TRANSPOSED: per c1-chunk, 16 accumulating
matmuls with the weight chunk stationary ([P(k),128c] x [P(k),8b] ->
[128c, 8b]) plus a K=1 bias matmul, all into one [P, CO, BL] PSUM bank,
then a single 140ns DVE cast to fp16.  No PE transposes and no per-chunk
PSUM->SBUF copies -- the old kernel's q phase kept DVE/ACT (the binding
engines at ~63us/rep) busy with copy work and plateaued at 67us/rep.

Scores are computed REPLICATED across all 128 partitions (the stationary
q column is free-dim-broadcast to 128 identical columns; moving cost is
unchanged), so reduce_max / Exp / reciprocal all produce per-partition
results directly and the two GpSimd partition_broadcasts per sample of
the old pipeline disappear -- freeing GpSimd for the q-phase copies and
the output store.

Cross-rep pipeline (emission order per rep r):
    loads(r) [wt + in2t + fcb on the ACT ring] -> phases 2-4 of rep
    r-1, with rep r's q-matmul chunk for c1-chunk b interleaved after
    sample b (filling the PE's idle gaps between score chains while
    rep r-1's x still streams) -> x loads(r) [sync ring, exclusive].
Thus rep r's weight load and whole q chain execute during rep r-1's
window, scores(r) can start the moment window r opens, and the sync
ring streams x back-to-back across reps (the output store rides the
GpSimd queue, so it never stalls the x stream the way the old kernel's
sync-ring store did).  (x tile slots rotate through a 16-deep pool;
emitting phases234(r-1) before x(r) keeps the pool's WAR tracking
exact.)

Per-sample phases 2-4 (software-pipelined: pool(b-1) after softmax(b)):
  2. scores: 16 M=128-replicated TensorE matmuls accumulating over the
     8 C1-chunks into one [P, 2, 392] PSUM tile.
  3. softmax: DVE negated reduce_max -> [P,1]; one ACT Exp (bias=-max,
     accum_out=sum) writing the fp16 attn row replicated [P, 784]; DVE
     reciprocal -> [P,1].
  4. pooling per C1-chunk, HW-A/B-tuned split: 6 chunks DVE
     tensor_tensor mult (2x fp16) + ACT Copy(scale=1/sum, accum_out);
     the last 2 chunks as one paired DVE mult [P,2,HW] + one axis-X
     paired reduce + a tiny normalize (measured cheaper than two 1x
     scalar_tensor_tensors; (7,1) and (5,3) splits measured 79/70us
     per rep vs 66 for this arrangement -- DVE and ACT are co-binding
     at ~8.2us/sample while the pure DMA stream is only 55us/rep).
"""

import numpy as np

import concourse.bacc as bacc
import concourse.mybir as mybir
import concourse.tile as tile
from concourse.bass_utils import run_bass_kernel_spmd

F32 = mybir.dt.float32
F16 = mybir.dt.float16

B, C1, C2, HW = 64, 1024, 2048, 784
NCORES = 8
BL = B // NCORES          # samples per core
P = 128                   # partitions
CO = C1 // P              # 8 c1 chunks
KC = C2 // P              # 16 c2 chunks
HH = HW // 2              # 392, half the pixels
XH = 2                    # x DMA split: halves of the c1-chunks per sample
COH = CO // XH            # c1-chunks per x half-tile
NTT = 5                   # pooling chunks: DVE tensor_tensor + ACT accum
NST = 2                   # pooling chunks: DVE scalar_tensor_tensor
SMBUFS = 6                # softmax/pooling small-tile pool depth
WMBUFS = 4                # wm (DVE mult output) rotation depth
WABUFS = 2                # wa (ACT accumulate elementwise out) depth
SPBUFS = 3                # scores PSUM tile depth (2 banks each)

_CACHE = {}


def _build(repeat=1):
    nc = bacc.Bacc(
        "TRN2", target_bir_lowering=False, debug=False, num_devices=NCORES
    )

    x = nc.dram_tensor("x", [BL, P, CO * HW], F16, kind="ExternalInput").ap()
    wt = nc.dram_tensor("wt", [C2, C1], F16, kind="ExternalInput").ap()
    in2t = nc.dram_tensor("in2t", [P, KC * BL], F16,
                          kind="ExternalInput").ap()
    fcb = nc.dram_tensor("fcb", [1, C1], F16, kind="ExternalInput").ap()
    out = nc.dram_tensor("out", [P, BL * CO], F32, kind="ExternalOutput").ap()
    with tile.TileContext(nc) as tc:
        _emit(tc, nc, x, wt, in2t, fcb, out, repeat=repeat)

    nc.compile()
    return nc


def _emit(tc, nc, x, wt, in2t, fcb, out, repeat=1):
    import contextlib

    ctx = contextlib.ExitStack()
    with ctx:
        const = ctx.enter_context(tc.tile_pool(name="const", bufs=1))
        wtp = ctx.enter_context(tc.tile_pool(name="wtp", bufs=2))
        xp = ctx.enter_context(tc.tile_pool(name="xp", bufs=BL * XH))
        sm = ctx.enter_context(tc.tile_pool(name="sm", bufs=SMBUFS))
        q_pp = ctx.enter_context(
            tc.tile_pool(name="q_pp", bufs=1, space="PSUM")
        )
        s_pp = ctx.enter_context(
            tc.tile_pool(name="s_pp", bufs=SPBUFS, space="PSUM")
        )
        wa_pp = ctx.enter_context(
            tc.tile_pool(name="wa_pp", bufs=1, space="PSUM")
        )

        ones_sb = const.tile([1, BL], F16, name="ones_sb", tag="ones_sb")
        nc.vector.memset(ones_sb[:], 1.0)

        xr = x.rearrange("b p (h c q) -> b p h c q", h=XH, c=COH)
        wtr = wt.rearrange("(k p) c -> p k c", p=P)

        prev = None
        for rep in range(repeat):
            # ---- loads for this rep (ACT HWDGE ring) ---------------------
            in2t_sb = wtp.tile([P, KC * BL], F16, name="in2t_sb",
                               tag="in2t_sb")
            nc.scalar.dma_start(out=in2t_sb[:], in_=in2t)
            fcb_sb = wtp.tile([1, C1], F16, name="fcb_sb", tag="fcb_sb")
            nc.scalar.dma_start(out=fcb_sb[:], in_=fcb)
            # wt rides the sync ring AHEAD of x, one 525KB slab per
            # c1-chunk, so interleaved q-matmul chunk co fires as soon as
            # its slab lands (~1.5us * (co+1) into the previous window)
            wt_sb = wtp.tile([P, KC, C1], F16, name="wt_sb", tag="wt_sb")
            for co in range(CO):
                cs = slice(co * P, (co + 1) * P)
                nc.sync.dma_start(out=wt_sb[:, :, cs], in_=wtr[:, :, cs])

            # ---- q chain for this rep: qT computed directly --------------
            # emitted as 8 per-c1-chunk closures, interleaved into the
            # previous rep's sample loop to fill PE idle gaps
            q_ps = q_pp.tile([P, CO, BL], F32, name="q_ps", tag="q_ps")
            qt_all = wtp.tile([P, CO, BL], F16, name="qt_all", tag="qt_all")

            def _mk_qmm(co, q_ps=q_ps, in2t_sb=in2t_sb, fcb_sb=fcb_sb,
                        wt_sb=wt_sb):
                def emit():
                    for k in range(KC):
                        nc.tensor.matmul(
                            q_ps[:, co, :],
                            wt_sb[:, k, co * P:(co + 1) * P],
                            in2t_sb[:, k * BL:(k + 1) * BL],
                            start=(k == 0),
                            stop=False,
                        )
                    nc.tensor.matmul(
                        q_ps[:, co, :],
                        fcb_sb[0:1, co * P:(co + 1) * P],
                        ones_sb[0:1, 0:BL],
                        start=False,
                        stop=True,
                    )
                return emit

            def _qcast(q_ps=q_ps, qt_all=qt_all):
                nc.vector.tensor_copy(qt_all[:], q_ps[:])

            qmm = [_mk_qmm(co) for co in range(CO)] + [_qcast]

            # ---- phases 2-4 of the PREVIOUS rep --------------------------
            if prev is not None:
                _emit_body(tc, nc, s_pp, sm, wtp, out, *prev, qmm=qmm,
                           wa_pp=wa_pp)
            else:
                for f in qmm:
                    f()

            # ---- x loads for this rep (sync ring, exclusive) -------------
            x_sb = []
            for b in range(BL):
                halves = []
                for h in range(XH):
                    t = xp.tile([P, COH, HW], F16, name="x_sb", tag="x_sb")
                    nc.sync.dma_start(out=t[:], in_=xr[b, :, h])
                    halves.append(t)
                x_sb.append(halves)

            prev = (x_sb, qt_all)

        _emit_body(tc, nc, s_pp, sm, wtp, out, *prev, qmm=[], wa_pp=wa_pp)


def _emit_body(tc, nc, s_pp, sm, wtp, out, x_sb, qt_all, qmm=(),
               wa_pp=None):
    # per-sample scores/softmax/pooling, software-pipelined, then store
    gall = wtp.tile([P, BL * CO], F32, name="gall", tag="gall")
    atiles = {}
    for b in range(BL):
        atiles[("s_ps", b)] = _emit_scores(nc, s_pp, x_sb, qt_all, b)
        if b < len(qmm):
            qmm[b]()       # next rep's q matmuls fill this PE idle gap
        _emit_softmax(nc, sm, b, atiles)
        # grouped mults one sample back; ACT accumulation two samples
        # back, so the coarse [P,4,HW] group products are always a full
        # cycle old when the ACT Copies read them (no phasing stall)
        if b >= 1:
            _emit_pool_mult(nc, sm, x_sb, gall, b - 1, atiles)
        if b >= 2:
            _emit_pool_accum(nc, sm, gall, b - 2, atiles)
        # reciprocal last: it waits on the ACT Exp, and the in-order DVE
        # must not head-block pooling work behind it
        _emit_recip(nc, sm, b, atiles)
    _emit_pool_mult(nc, sm, x_sb, gall, BL - 1, atiles)
    _emit_pool_accum(nc, sm, gall, BL - 2, atiles)
    _emit_pool_accum(nc, sm, gall, BL - 1, atiles)
    for f in qmm[BL:]:
        f()
    # store from the GpSimd queue: it has no per-sample work, so a waiting
    # dma_start never blocks anything on the critical path
    nc.gpsimd.dma_start(out=out, in_=gall[:])


def _emit_scores(nc, s_pp, x_sb, qt_all, b):
    # 16 matmuls; the stationary q column is broadcast to 128 identical
    # columns so every output partition carries the same score row
    # [P, 2, 512] f32 = 4KB/partition = exactly 2 PSUM banks, so each
    # 392-wide half-slice stays inside its own bank
    s_ps = s_pp.tile([P, 2, 512], F32, name="s_ps", tag="s_ps")
    for co in range(CO):
        xt = x_sb[b][co // COH]
        stat = qt_all[:, co, b:b + 1].broadcast_to((P, P))
        for h in range(2):
            nc.tensor.matmul(
                s_ps[:, h, 0:HH],
                stat,
                xt[:, co % COH, h * HH:(h + 1) * HH],
                start=(co == 0),
                stop=(co == CO - 1),
            )
    return s_ps


def _emit_softmax(nc, sm, b, atiles):
    s_ps = atiles.pop(("s_ps", b))
    nm = sm.tile([P, 1], F32, name="nm", tag="nm")
    nc.vector.tensor_reduce(
        nm[:], s_ps[:, :, 0:HH], axis=mybir.AxisListType.XY,
        op=mybir.AluOpType.max, negate=True,
    )
    l = sm.tile([P, 1], F32, name="l", tag="l")
    a_sb = sm.tile([P, HW], F16, name="a_sb", tag="a_sb")
    nc.scalar.activation(
        a_sb.rearrange("p (h n) -> p h n", h=2),
        s_ps[:, :, 0:HH],
        mybir.ActivationFunctionType.Exp,
        bias=nm[:], accum_out=l[:],
    )
    atiles[("a", b)] = a_sb
    atiles[("l", b)] = l


def _emit_recip(nc, sm, b, atiles):
    l = atiles.pop(("l", b))
    r_bc = sm.tile([P, 1], F32, name="r_bc", tag="r_bc")
    nc.vector.reciprocal(r_bc[:], l[:])
    atiles[("r", b)] = r_bc


def _emit_pool_mult(nc, sm, x_sb, gall, b, atiles):
    # one grouped DVE mult per x half (2x fp16, [P, COH, HW]); the last
    # NG chunks also get their axis-X grouped reduce here (unnormalized)
    a_sb = atiles.pop(("a", b))
    wm4s = []
    for h in range(XH):
        wm4 = sm.tile([P, COH, HW], F16, name="wm4", tag="wm4", bufs=4)
        a4 = a_sb[:].unsqueeze(1).broadcast_to((P, COH, HW))
        nc.vector.tensor_tensor(
            out=wm4[:], in0=x_sb[b][h][:], in1=a4,
            op=mybir.AluOpType.mult,
        )
        wm4s.append(wm4)
    NG = CO - NTT
    gu = sm.tile([P, NG], F32, name="gu", tag="gu", bufs=4)
    nc.vector.tensor_reduce(
        gu[:], wm4s[1][:, COH - NG:COH, :], axis=mybir.AxisListType.X,
        op=mybir.AluOpType.add,
    )
    atiles[("wm4", b)] = wm4s
    atiles[("gu", b)] = gu


def _emit_pool_accum(nc, sm, gall, b, atiles):
    # NTT chunks: ACT Copy(scale=1/sum, accum_out) reading group-product
    # slices; the NG reduced chunks: one ACT Copy normalize of gu
    wm4s = atiles.pop(("wm4", b))
    gu = atiles.pop(("gu", b))
    r_bc = atiles.pop(("r", b))
    wa = sm.tile([P, HW], F16, name="wa", tag="wa", bufs=WABUFS)
    for co in range(NTT):
        nc.scalar.activation(
            wa[:], wm4s[co // COH][:, co % COH, :],
            mybir.ActivationFunctionType.Copy,
            scale=r_bc[:],
            accum_out=gall[:, b * CO + co:b * CO + co + 1],
        )
    NG = CO - NTT
    nc.scalar.activation(
        gall[:, b * CO + NTT:b * CO + CO], gu[:],
        mybir.ActivationFunctionType.Copy,
        scale=r_bc[:],
    )


def _get_nc():
    if "nc" not in _CACHE:
        _CACHE["nc"] = _build()
    return _CACHE["nc"]


def _in_maps(input1, input2, fc_w, fc_b):
    input1 = np.asarray(input1, dtype=np.float32)
    input2 = np.asarray(input2, dtype=np.float32)
    fc_w = np.asarray(fc_w, dtype=np.float32)
    fc_b = np.asarray(fc_b, dtype=np.float32)

    wt = np.ascontiguousarray(fc_w.T.astype(np.float16))      # [C2, C1]
    fcb = np.ascontiguousarray(fc_b.reshape(1, C1).astype(np.float16))
    maps = []
    for i in range(NCORES):
        sl = slice(i * BL, (i + 1) * BL)
        # x[b, co*128+ci, q] -> [b, ci, co*HW+q]
        x_sh = np.ascontiguousarray(
            input1[sl]
            .reshape(BL, CO, P, HW)
            .transpose(0, 2, 1, 3)
            .reshape(BL, P, CO * HW)
            .astype(np.float16)
        )
        # in2t[p, k*BL + b] = input2[i*BL + b, k*128 + p]
        i2t = np.ascontiguousarray(
            input2[sl].T.reshape(KC, P, BL)
            .transpose(1, 0, 2)
            .reshape(P, KC * BL)
            .astype(np.float16)
        )
        maps.append({"x": x_sh, "wt": wt, "in2t": i2t, "fcb": fcb})
    return maps


def _assemble(results):
    outs = []
    for i in range(NCORES):
        arr = np.asarray(results[i]["out"])                 # [128, BL*CO]
        # arr[ci, b*CO + co] = g[b, co*128 + ci]
        outs.append(
            arr.reshape(P, BL, CO).transpose(1, 2, 0).reshape(BL, C1)
        )
    return np.ascontiguousarray(
        np.concatenate(outs, axis=0).astype(np.float32)
    )


def run(input1, input2, fc_w, fc_b, trace=False, **trace_kwargs):
    nc = _get_nc()
    res = run_bass_kernel_spmd(
        nc,
        _in_maps(input1, input2, fc_w, fc_b),
        core_ids=list(range(NCORES)),
        trace=trace,
        **trace_kwargs,
    )
    return _assemble(res.results), res


def kernel(input1, input2, fc_w, fc_b):
    out, _ = run(input1, input2, fc_w, fc_b)
    return out
